# revision 1
# baseline (speedup 1.0000x reference)
"""GCN link predictor on 8 Trainium2 NeuronCores — v3.

Structure (descriptor-cost-optimized; DMA descriptors dominate on this part):
  - Nodes permuted + binned into 8*49 blocks of 128 (balanced by in-degree),
    partitioned across cores by contiguous block ranges.
  - Scatter-add over edges via PE matmuls accumulating the TRANSPOSED block
    output aggT[f, dloc] = sum_e gt[e, f] * M[e, dloc]; the dense layer
    weight applies post-aggregation (one matmul per block) so there are no
    standalone dense phases.
  - M tiles are host-built one-hot (bandwidth is free here), streamed per
    chunk on the Act engine's DMA queue.
  - Self-loops via identity matmul on SBUF-retained local blocks (no gather
    slots, no extra DMA).
  - Per-edge source rows fetched with gpsimd dma_gather from the AllGathered
    node table; EVERY gather call is split across SWDGE queues (descriptor
    processing parallelizes across queues: ~0.8ns/desc vs 6.9 serial).
  - Decode: data-parallel edges, transposed row-gathers of z2 (split across
    queues) + PE dot products; scores accumulate into one SBUF tile, one
    output DMA per kernel.
"""

import sys

for _p in ("/opt/trn_rl_repo",):
    if _p not in sys.path:
        sys.path.insert(0, _p)

import heapq

import numpy as np
import ml_dtypes

P = 128
NCORE = 8
NB = 49                 # dst blocks per core
SHARD = NB * P          # 6272 node slots per core
NTAB = NCORE * SHARD    # 50176 table rows
SPLIT = NTAB // 2       # 25088, int16-addressable halves
D = 128
CHB = 7                 # blocks per gather chunk
NCHUNK = NB // CHB
N_NODES = 50000
EC = 12500              # decode edges per core per kind
NGRP = 8                # gpsimd 16-partition groups
_G = -(-EC // NGRP)
NDEC = -(-_G // 16) * 16  # padded per-group decode slots
NJ = SHARD // 16        # 392 u/v cols per core per partition
NE16 = NCORE * NJ       # 3136 table cols per partition

BF16 = ml_dtypes.bfloat16


def _wrap_idx(a):
    """dma_gather index layout: element j at [j%16, j//16], replicated to 128 partitions."""
    assert a.shape[0] % 16 == 0
    w = a.reshape(-1, 16).T.astype(np.int16)
    return np.ascontiguousarray(np.tile(w, (8, 1)))


def _balance_nodes(indeg):
    """Greedy bin packing: 392 bins of 128 slots, balancing summed in-degree."""
    nbins = NCORE * NB
    order = np.argsort(-indeg, kind="stable")
    space = [P] * nbins
    heap = [(0, b) for b in range(nbins)]
    heapq.heapify(heap)
    assign = np.empty(N_NODES, np.int64)
    for n in order:
        while True:
            load, b = heapq.heappop(heap)
            if space[b] > 0:
                break
        assign[n] = b
        space[b] -= 1
        if space[b] > 0:
            heapq.heappush(heap, (load + int(indeg[n]), b))
    row_of_node = np.empty(N_NODES, np.int64)
    node_of_row = np.full(NTAB, -1, np.int64)
    fill = np.zeros(nbins, np.int64)
    for n in range(N_NODES):
        b = assign[n]
        s = b * P + fill[b]
        fill[b] += 1
        row_of_node[n] = s
        node_of_row[s] = n
    return row_of_node, node_of_row


def _within_group(sort_idx, gid):
    """Position of each element within its (sorted) group."""
    g = gid[sort_idx]
    n = len(g)
    starts = np.r_[0, np.flatnonzero(np.diff(g)) + 1]
    lens = np.diff(np.r_[starts, n])
    within = np.arange(n) - np.repeat(starts, lens)
    out = np.empty(n, np.int64)
    out[sort_idx] = within
    return out


def prepare(inputs):
    x = np.asarray(inputs["x"], np.float32)
    ei = np.asarray(inputs["edge_index"], np.int64)
    ew = np.asarray(inputs["edge_weight"], np.float32)
    pe = np.asarray(inputs["pos_edge_index"], np.int64)
    ne = np.asarray(inputs["neg_edge_index"], np.int64)
    W1 = np.asarray(inputs["W1"], np.float32)
    b1 = np.asarray(inputs["b1"], np.float32)
    W2 = np.asarray(inputs["W2"], np.float32)
    b2 = np.asarray(inputs["b2"], np.float32)
    Wl = np.asarray(inputs["W_link"], np.float32)
    bl = np.asarray(inputs["b_link"], np.float32)

    src, dst = ei[0], ei[1]
    rdeg = np.bincount(dst, minlength=N_NODES)
    row_of_node, node_of_row = _balance_nodes(rdeg)

    # ---- 2nd pass: rebalance (lo, hi) in-edge counts per bin, within halves ----
    half_of_src = (row_of_node[src] >= SPLIT).astype(np.int64)
    cnt_lo = np.bincount(dst[half_of_src == 0], minlength=N_NODES)
    cnt_hi = np.bincount(dst[half_of_src == 1], minlength=N_NODES)
    self_half = row_of_node >= SPLIT
    nbins_half = NCORE * NB // 2
    new_row = np.empty(N_NODES, np.int64)
    for hsel, base in ((~self_half, 0), (self_half, SPLIT)):
        nodes = np.flatnonzero(hsel)
        w = cnt_lo[nodes] + cnt_hi[nodes]
        order = nodes[np.argsort(-w, kind="stable")]
        loads = np.zeros((nbins_half, 2), np.float64)
        space = np.full(nbins_half, P, np.int64)
        fill = np.zeros(nbins_half, np.int64)
        for n in order:
            cl, ch = cnt_lo[n], cnt_hi[n]
            score = np.maximum(loads[:, 0] + cl, loads[:, 1] + ch) \
                + 0.5 * (loads[:, 0] + loads[:, 1])
            score[space == 0] = np.inf
            b = int(np.argmin(score))
            loads[b, 0] += cl
            loads[b, 1] += ch
            space[b] -= 1
            new_row[n] = base + b * P + fill[b]
            fill[b] += 1
    row_of_node = new_row
    node_of_row = np.full(NTAB, -1, np.int64)
    node_of_row[row_of_node] = np.arange(N_NODES)

    # ---- edge slots: real edges only (self-loops handled locally) ----
    rs = row_of_node[src]
    rd = row_of_node[dst]
    core = rd // SHARD
    blk = (rd % SHARD) // P
    dloc = rd % P
    half = (rs >= SPLIT).astype(np.int64)
    locsrc = rs - half * SPLIT

    cnt = np.zeros((NCORE, NB, 2), np.int64)
    np.add.at(cnt, (core, blk, half), 1)
    K = -(-cnt.max(axis=0) // P)  # [NB, 2]; may be 0

    tb = np.zeros((NB, 2), np.int64)
    chunk_info = []
    pos_t = 0
    for c in range(NCHUNK):
        lo_base = pos_t
        for b in range(c * CHB, (c + 1) * CHB):
            tb[b, 0] = pos_t
            pos_t += K[b, 0]
        lo_n = pos_t - lo_base
        hi_base = pos_t
        for b in range(c * CHB, (c + 1) * CHB):
            tb[b, 1] = pos_t
            pos_t += K[b, 1]
        hi_n = pos_t - hi_base
        chunk_info.append((lo_base, lo_n, hi_base, hi_n))
    TTOT = pos_t
    S = TTOT * P

    # ---- slot assignment ----
    sidx = np.lexsort((half, blk, core))
    gid = (core * NB + blk) * 2 + half
    within = _within_group(sidx, gid)
    slot = tb[blk, half] * P + within

    # ---- per-core M (partition-major) and gather indices ----
    mw_list, gidx_list = [], []
    for c in range(NCORE):
        sel = core == c
        m = np.zeros((S, P), np.float32)
        m[slot[sel], dloc[sel]] = ew[sel]
        m = m.astype(BF16).reshape(TTOT, P, P).transpose(1, 0, 2).reshape(P, S)
        mw_list.append(np.ascontiguousarray(m))
        g = np.zeros(S, np.int64)
        g[slot[sel]] = locsrc[sel]
        gidx_list.append(_wrap_idx(g))

    # ---- EWD: per-dst-slot edge weights (incl self-loops) for f32 degrees ----
    loop = np.arange(N_NODES)
    dstL = np.concatenate([dst, loop])
    ewL = np.concatenate([ew, np.ones(N_NODES, np.float32)])
    rdL = row_of_node[dstL]
    coreL = rdL // SHARD
    blkL = (rdL % SHARD) // P
    dlocL = rdL % P
    Wdeg = int((rdeg + 1).max())
    sidx2 = np.lexsort((dlocL, blkL, coreL))
    gid2 = (coreL * NB + blkL) * P + dlocL
    within2 = _within_group(sidx2, gid2)
    ewd_list = []
    for c in range(NCORE):
        sel = coreL == c
        e = np.zeros((P, NB * Wdeg), np.float32)
        e[dlocL[sel], blkL[sel] * Wdeg + within2[sel]] = ewL[sel]
        ewd_list.append(e)

    # ---- masks and node-major features ----
    valid = (node_of_row >= 0).astype(np.float32)
    mask_list, imask_list, xn_list = [], [], []
    xfull = np.zeros((NTAB, D), np.float32)
    vsel = node_of_row >= 0
    xfull[vsel] = x[node_of_row[vsel]]
    for c in range(NCORE):
        v = valid[c * SHARD:(c + 1) * SHARD].reshape(NB, P)
        mask_list.append(np.ascontiguousarray(v.T))
        imask_list.append(np.ascontiguousarray(1.0 - v.T))
        xn_list.append(np.ascontiguousarray(xfull[c * SHARD:(c + 1) * SHARD]))

    # ---- decode: group assignment + ap_gather indices + partition masks ----
    book = {}
    didx = {}
    dmask = {}
    for kind, e in (("pos", pe), ("neg", ne)):
        r0 = row_of_node[e[0]]
        r1 = row_of_node[e[1]]
        for c in range(NCORE):
            ids = np.arange(c * EC, (c + 1) * EC)
            gg = np.arange(EC) % NGRP
            jj = np.arange(EC) // NGRP
            bk = np.full((NGRP, NDEC), -1, np.int64)
            bk[gg, jj] = ids
            book[(kind, c)] = bk
            for nm, rr in (("u", r0[ids]), ("v", r1[ids])):
                iv = np.zeros((NGRP, NDEC), np.int64)
                iv[gg, jj] = (rr // SHARD) * NJ + (rr % SHARD) // 16
                ix = np.zeros((P, NDEC // 16), np.int16)
                for g in range(NGRP):
                    ix[16 * g:16 * (g + 1), :] = \
                        iv[g].reshape(-1, 16).T.astype(np.int16)
                didx[(kind, c, nm)] = np.ascontiguousarray(ix)
                mk = np.zeros((P, NDEC), np.float32)
                mk[16 * gg + (rr % 16), jj] = 1.0
                dmask[(kind, c, nm)] = np.ascontiguousarray(mk.astype(BF16))

    b1t = np.ascontiguousarray(np.tile(b1.reshape(1, D), (P, 1)))
    w2t = np.ascontiguousarray(W2.T)
    wl2 = np.ascontiguousarray(np.concatenate([Wl[:D, :], Wl[D:, :]], axis=1))
    b2c = np.ascontiguousarray(b2.reshape(D, 1))
    blb = np.full((P, 1), float(bl[0]), np.float32)
    gsel = np.zeros((P, NGRP), np.float32)
    gsel[np.arange(P), np.arange(P) // 16] = 1.0

    meta = dict(
        S=S, TTOT=TTOT, K=K, tb=tb, chunk_info=chunk_info, Wdeg=Wdeg,
        book=book, npos=pe.shape[1], nneg=ne.shape[1],
    )

    in_maps = []
    for c in range(NCORE):
        in_maps.append({
            "mw": mw_list[c], "gidx": gidx_list[c], "ewd": ewd_list[c],
            "mask": mask_list[c], "imask": imask_list[c], "xn": xn_list[c],
            "w1": W1, "w2t": w2t, "wl2": wl2, "b2c": b2c,
            "b1t": b1t, "blb": blb, "gsel": gsel,
            "uidxp": didx[("pos", c, "u")], "vidxp": didx[("pos", c, "v")],
            "uidxn": didx[("neg", c, "u")], "vidxn": didx[("neg", c, "v")],
            "umaskp": dmask[("pos", c, "u")], "vmaskp": dmask[("pos", c, "v")],
            "umaskn": dmask[("neg", c, "u")],
            "vmaskn": dmask[("neg", c, "v")],
        })
    return meta, in_maps


def build(meta, reps=1, sim_mode=False, ablate=frozenset()):
    import concourse.bacc as bacc
    import concourse.tile as tile
    import concourse.mybir as mybir

    f32 = mybir.dt.float32
    bf16 = mybir.dt.bfloat16
    i16 = mybir.dt.int16
    i32 = mybir.dt.int32

    S = meta["S"]
    K = meta["K"]
    tb = meta["tb"]
    chunk_info = meta["chunk_info"]
    Wdeg = meta["Wdeg"]

    nc = bacc.Bacc("TRN2", target_bir_lowering=False, debug=False,
                   num_devices=NCORE, num_swdge_queues=4)

    mw = nc.dram_tensor("mw", [P, S], bf16, kind="ExternalInput")
    gidx = nc.dram_tensor("gidx", [P, S // 16], i16, kind="ExternalInput")
    ewd = nc.dram_tensor("ewd", [P, NB * Wdeg], f32, kind="ExternalInput")
    maskd = nc.dram_tensor("mask", [P, NB], f32, kind="ExternalInput")
    imaskd = nc.dram_tensor("imask", [P, NB], f32, kind="ExternalInput")
    xnd = nc.dram_tensor("xn", [SHARD, D], f32, kind="ExternalInput")
    w1d = nc.dram_tensor("w1", [D, D], f32, kind="ExternalInput")
    w2td = nc.dram_tensor("w2t", [D, D], f32, kind="ExternalInput")
    wl2d = nc.dram_tensor("wl2", [D, 2], f32, kind="ExternalInput")
    b2cd = nc.dram_tensor("b2c", [D, 1], f32, kind="ExternalInput")
    b1d = nc.dram_tensor("b1t", [P, D], f32, kind="ExternalInput")
    blbd = nc.dram_tensor("blb", [P, 1], f32, kind="ExternalInput")
    gseld = nc.dram_tensor("gsel", [P, NGRP], f32, kind="ExternalInput")
    dix = {}
    for kind in ("pos", "neg"):
        for nm in ("u", "v"):
            dix[(kind, nm, "i")] = nc.dram_tensor(
                f"{nm}idx{kind[0]}", [P, NDEC // 16], i16,
                kind="ExternalInput")
            dix[(kind, nm, "m")] = nc.dram_tensor(
                f"{nm}mask{kind[0]}", [P, NDEC], bf16, kind="ExternalInput")
    scored = nc.dram_tensor("scores", [2 * NGRP, NDEC], f32,
                            kind="ExternalOutput")

    Copy = mybir.ActivationFunctionType.Copy
    Add = mybir.AluOpType.add
    Mult = mybir.AluOpType.mult
    IsEq = mybir.AluOpType.is_equal
    Max = mybir.AluOpType.max
    AG = "AllGather"
    rg = [list(range(NCORE))]

    def qsplit_gather(gt, toff, tn, tab_half, idx_off, q0, q1):
        """Split one half-gather across two SWDGE queues."""
        ta = tn // 2
        for tq, tw, q in ((0, ta, q0), (ta, tn - ta, q1)):
            if tw == 0:
                continue
            o = idx_off + tq * 8
            nc.gpsimd.dma_gather(
                gt[:, toff + tq:toff + tq + tw, :], tab_half,
                gidx_t[:, o:o + tw * 8],
                tw * P, tw * P, D, single_packet=False, queue_num=q)

    with tile.TileContext(nc) as tc:
        with tc.tile_pool(name="const", bufs=1) as cp, \
             tc.tile_pool(name="dram", bufs=1, space="DRAM") as dram:

            # ---------- constants ----------
            gidx_t = cp.tile([P, S // 16], i16)
            nc.sync.dma_start(out=gidx_t[:], in_=gidx[:])

            ewd_t = cp.tile([P, NB, Wdeg], f32)
            nc.sync.dma_start(out=ewd_t[:],
                              in_=ewd[:].rearrange("p (b w) -> p b w",
                                                   w=Wdeg))
            mask_t = cp.tile([P, NB], f32)
            nc.sync.dma_start(out=mask_t[:], in_=maskd[:])
            imask_t = cp.tile([P, NB], f32)
            nc.sync.dma_start(out=imask_t[:], in_=imaskd[:])

            w1f = cp.tile([D, D], f32)
            nc.sync.dma_start(out=w1f[:], in_=w1d[:])
            w1b = cp.tile([D, D], bf16)
            nc.vector.tensor_copy(out=w1b[:], in_=w1f[:])
            w2tf = cp.tile([D, D], f32)
            nc.sync.dma_start(out=w2tf[:], in_=w2td[:])
            w2tb = cp.tile([D, D], bf16)
            nc.vector.tensor_copy(out=w2tb[:], in_=w2tf[:])
            wl2f = cp.tile([D, 2], f32)
            nc.sync.dma_start(out=wl2f[:], in_=wl2d[:])
            wl2b = cp.tile([D, 2], bf16)
            nc.vector.tensor_copy(out=wl2b[:], in_=wl2f[:])
            b2cf = cp.tile([D, 1], f32)
            nc.sync.dma_start(out=b2cf[:], in_=b2cd[:])
            b2cb = cp.tile([D, 1], bf16)
            nc.vector.tensor_copy(out=b2cb[:], in_=b2cf[:])
            b1t_t = cp.tile([P, D], f32)
            nc.sync.dma_start(out=b1t_t[:], in_=b1d[:])
            blb_t = cp.tile([P, 1], f32)
            nc.sync.dma_start(out=blb_t[:], in_=blbd[:])
            gself = cp.tile([P, NGRP], f32)
            nc.sync.dma_start(out=gself[:], in_=gseld[:])
            gselb = cp.tile([P, NGRP], bf16)
            nc.vector.tensor_copy(out=gselb[:], in_=gself[:])
            didx_t = {}
            for kind in ("pos", "neg"):
                for nm in ("u", "v"):
                    it = cp.tile([P, NDEC // 16], i16, name=f"dix_{nm}{kind}")
                    nc.sync.dma_start(out=it[:], in_=dix[(kind, nm, "i")][:])
                    mk_ = cp.tile([P, NDEC], bf16, name=f"dmk_{nm}{kind}")
                    nc.sync.dma_start(out=mk_[:], in_=dix[(kind, nm, "m")][:])
                    didx_t[(kind, nm)] = (it, mk_)

            # identity (bf16) for self-loop matmuls
            idi = cp.tile([P, P], i32)
            nc.gpsimd.iota(out=idi[:], pattern=[[1, P]], base=0,
                           channel_multiplier=-1)
            identb = cp.tile([P, P], bf16)
            nc.vector.tensor_scalar(out=identb[:], in0=idi[:], scalar1=0,
                                    scalar2=None, op0=IsEq)

            # ---------- w2uv = W2 @ [wl_top | wl_bot], Cb = b2.(wlt+wlb)+bl --
            w2uv = cp.tile([D, 2], bf16)
            Cb = cp.tile([NGRP, 1], f32)
            with tc.tile_pool(name="cpsum", bufs=1, space="PSUM") as cpp:
                pw = cpp.tile([D, 2], f32, space="PSUM", tag="pw")
                nc.tensor.matmul(out=pw[:], lhsT=w2tb[:], rhs=wl2b[:],
                                 start=True, stop=True)
                nc.scalar.copy(out=w2uv[:], in_=pw[:])
                pc = cpp.tile([1, 2], f32, space="PSUM", tag="pc")
                nc.tensor.matmul(out=pc[:], lhsT=b2cb[:], rhs=wl2b[:],
                                 start=True, stop=True)
                cuv_s = cp.tile([1, 2], f32)
                nc.scalar.copy(out=cuv_s[:], in_=pc[:])
                c_s = cp.tile([1, 1], f32)
                nc.vector.tensor_reduce(out=c_s[:], in_=cuv_s[:],
                                        axis=mybir.AxisListType.X, op=Add)
                cT = cp.tile([1, 1], f32)
                nc.vector.tensor_tensor(out=cT[:], in0=c_s[:],
                                        in1=blb_t[0:1, :], op=Add)
                ones18 = cp.tile([1, NGRP], f32)
                nc.vector.memset(ones18[:], 1.0)
                pb = cpp.tile([NGRP, 1], f32, space="PSUM", tag="pb")
                nc.tensor.matmul(out=pb[:], lhsT=ones18[:], rhs=cT[:],
                                 start=True, stop=True)
                nc.scalar.copy(out=Cb[:], in_=pb[:])

            # ---------- degree -> dinv ----------
            deg = cp.tile([P, NB], f32)
            nc.vector.tensor_reduce(out=deg[:], in_=ewd_t[:],
                                    axis=mybir.AxisListType.X, op=Add)
            nc.vector.tensor_tensor(out=deg[:], in0=deg[:], in1=imask_t[:],
                                    op=Add)
            rec = cp.tile([P, NB], f32)
            nc.vector.reciprocal(out=rec[:], in_=deg[:])
            dinv = cp.tile([P, NB], f32)
            nc.scalar.sqrt(out=dinv[:], in_=rec[:])
            nc.vector.tensor_tensor(out=dinv[:], in0=dinv[:], in1=mask_t[:],
                                    op=Mult)
            # dinvinv = sqrt(deg) (masked), flattened to a partition-0 row:
            # per-block rank-1 bias matmul operand folded into the W1 chain
            dvi = cp.tile([P, NB], f32)
            nc.scalar.sqrt(out=dvi[:], in_=deg[:])
            nc.vector.tensor_tensor(out=dvi[:], in0=dvi[:], in1=mask_t[:],
                                    op=Mult)
            dvib = cp.tile([P, NB], bf16)
            nc.vector.tensor_copy(out=dvib[:], in_=dvi[:])
            dinvinvT = cp.tile([NB, P], f32)
            with tc.tile_pool(name="dpsum", bufs=1, space="PSUM") as dpp:
                pt = dpp.tile([NB, P], f32, space="PSUM", tag="pt")
                nc.tensor.matmul(out=pt[:], lhsT=dvib[:], rhs=identb[:],
                                 start=True, stop=True)
                nc.scalar.copy(out=dinvinvT[:], in_=pt[:])
            dTs = dram.tile([NB, P], f32, name="dTscratch")
            nc.sync.dma_start(out=dTs[:], in_=dinvinvT[:])
            dTflat = cp.tile([1, NB * P], f32)
            nc.sync.dma_start(
                out=dTflat[:],
                in_=dTs[:].rearrange("b p -> (b p)").unsqueeze(0))

            for _rep in range(reps):
                xloc = dram.tile([SHARD, D], bf16, name=f"xloc{_rep}")
                xtab = dram.tile([NTAB, D], bf16, addr_space="Shared",
                                 name=f"xtab{_rep}")
                z1loc = dram.tile([SHARD, D], bf16, name=f"z1loc{_rep}")
                z1tab = dram.tile([NTAB, D], bf16, addr_space="Shared",
                                  name=f"z1tab{_rep}")
                uvloc = dram.tile([2, 16, NJ], f32, name=f"uvloc{_rep}")
                uvtab = dram.tile([2 * NCORE, 16, NJ], f32,
                                  addr_space="Shared", name=f"uvtab{_rep}")

                with tc.tile_pool(name=f"mp{_rep}", bufs=2) as mpool, \
                     tc.tile_pool(name=f"gp{_rep}", bufs=2) as gpool, \
                     tc.tile_pool(name=f"tp{_rep}", bufs=2) as tpool, \
                     tc.tile_pool(name=f"kp{_rep}", bufs=1) as kpool, \
                     tc.tile_pool(name=f"sp{_rep}", bufs=4,
                                  space="PSUM") as spsum, \
                     tc.tile_pool(name=f"hp{_rep}", bufs=2,
                                  space="PSUM") as hpsum:

                    xkeep = kpool.tile([P, NB, D], bf16, tag="xk")
                    uvkeep = kpool.tile([P, NB, 2], f32, tag="uvk")

                    # ---------- phase A: x\' = dinv * x (bf16) ----------
                    for ci in (() if "xphase" in ablate else range(NCHUNK)):
                        c0 = ci * CHB
                        xf = tpool.tile([P, CHB, D], f32, tag="xf")
                        nc.sync.dma_start(
                            out=xf[:],
                            in_=xnd[c0 * P:(c0 + CHB) * P, :]
                            .rearrange("(b n) f -> n b f", n=P))
                        for j in range(CHB):
                            b = c0 + j
                            nc.vector.tensor_scalar(
                                out=xkeep[:, b, :], in0=xf[:, j, :],
                                scalar1=dinv[:, b:b + 1], scalar2=None,
                                op0=Mult)
                        nc.sync.dma_start(
                            out=xloc[c0 * P:(c0 + CHB) * P, :]
                            .rearrange("(b n) f -> n b f", n=P),
                            in_=xkeep[:, c0:c0 + CHB, :])
                    if "xphase" in ablate:
                        nc.vector.memset(xkeep[:, 0, :], 0)
                    if not sim_mode:
                        nc.gpsimd.collective_compute(
                            AG, mybir.AluOpType.bypass, replica_groups=rg,
                            ins=[xloc[:]], outs=[xtab[:]])

                    def conv_pass(tab, keep, layer, zloc):
                        for ci, (lo_base, lo_n, hi_base, hi_n) in \
                                enumerate(chunk_info):
                            tot = lo_n + hi_n
                            c0 = ci * CHB
                            mt = mpool.tile([P, tot, P], bf16, tag="m")
                            if "mload" in ablate:
                                nc.vector.memset(mt[:, 0, :], 0)
                            else:
                                nc.scalar.dma_start(
                                    out=mt[:],
                                    in_=mw[:, lo_base * P:
                                           (lo_base + tot) * P]
                                    .rearrange("p (t f) -> p t f", f=P))
                            gt = gpool.tile([P, tot, D], bf16, tag="g")
                            if "gather" in ablate:
                                nc.vector.memset(gt[:, 0, :], 0)
                            else:
                                qsplit_gather(gt, 0, lo_n, tab[:SPLIT, :],
                                              lo_base * 8, 0, 1)
                                qsplit_gather(gt, lo_n, hi_n, tab[SPLIT:, :],
                                              hi_base * 8, 2, 3)
                            for b in range(c0, c0 + CHB):
                                sp = spsum.tile([P, D], f32, space="PSUM",
                                                tag="sp")
                                tl = [tb[b, 0] - lo_base + t
                                      for t in range(K[b, 0])]
                                tl += [lo_n + tb[b, 1] - hi_base + t
                                       for t in range(K[b, 1])]
                                if "matmul" in ablate:
                                    tl = tl[:1]
                                nc.tensor.matmul(out=sp[:],
                                                 lhsT=keep[:, b, :],
                                                 rhs=identb[:],
                                                 start=True,
                                                 stop=(len(tl) == 0))
                                for i, t in enumerate(tl):
                                    nc.tensor.matmul(out=sp[:],
                                                     lhsT=gt[:, t, :],
                                                     rhs=mt[:, t, :],
                                                     start=False,
                                                     stop=(i == len(tl) - 1))
                                if "postops" in ablate:
                                    continue
                                aggs = tpool.tile([P, D], bf16, tag="as",
                                                  bufs=4)
                                nc.scalar.copy(out=aggs[:], in_=sp[:])
                                if layer == 1:
                                    hp = hpsum.tile([P, D], f32,
                                                    space="PSUM", tag="hp")
                                    nc.tensor.matmul(out=hp[:], lhsT=aggs[:],
                                                     rhs=w1b[:],
                                                     start=True, stop=False)
                                    nc.tensor.matmul(
                                        out=hp[:],
                                        lhsT=dTflat[0:1,
                                                    b * P:(b + 1) * P],
                                        rhs=b1t_t[0:1, :],
                                        start=False, stop=True)
                                    t1 = tpool.tile([P, D], f32, tag="t1")
                                    nc.scalar.activation(
                                        out=t1[:], in_=hp[:],
                                        func=mybir.ActivationFunctionType
                                        .Relu,
                                        scale=dinv[:, b:b + 1])
                                    nc.vector.tensor_scalar(
                                        out=keep[:, b, :], in0=t1[:],
                                        scalar1=dinv[:, b:b + 1],
                                        scalar2=None, op0=Mult)
                                else:
                                    up = hpsum.tile([P, D], f32,
                                                    space="PSUM", tag="hp")
                                    nc.tensor.matmul(out=up[:, 0:2],
                                                     lhsT=aggs[:],
                                                     rhs=w2uv[:],
                                                     start=True, stop=True)
                                    nc.scalar.activation(
                                        out=uvkeep[:, b, :],
                                        in_=up[:, 0:2],
                                        func=Copy, scale=dinv[:, b:b + 1])
                            if layer == 1 and "zwrite" not in ablate and \
                                    "postops" not in ablate:
                                nc.sync.dma_start(
                                    out=zloc[c0 * P:(c0 + CHB) * P, :]
                                    .rearrange("(b n) f -> n b f", n=P),
                                    in_=keep[:, c0:c0 + CHB, :])

                    conv_pass(xtab, xkeep, 1, z1loc)
                    if "postops" in ablate:
                        nc.vector.memset(xkeep[:, 0, :], 0)
                        nc.vector.memset(uvkeep[:, 0, :], 0)
                    if not sim_mode:
                        nc.gpsimd.collective_compute(
                            AG, mybir.AluOpType.bypass, replica_groups=rg,
                            ins=[z1loc[:]], outs=[z1tab[:]])
                    conv_pass(z1tab, xkeep, 2, None)

                    # uvkeep [p=16t+s, b, q] -> uvloc[q, s, b*8+t]
                    if "zwrite" not in ablate and "postops" not in ablate:
                        uvw = uvloc[:].rearrange("q s (b t) -> s b q t",
                                                 t=8)
                        for t in range(8):
                            nc.sync.dma_start(
                                out=uvw[:, :, :, t:t + 1],
                                in_=uvkeep[16 * t:16 * (t + 1), :, :]
                                .unsqueeze(3))
                    if not sim_mode:
                        nc.gpsimd.collective_compute(
                            AG, mybir.AluOpType.bypass, replica_groups=rg,
                            ins=[uvloc[:]], outs=[uvtab[:]])

                # ---------- decode ----------
                with tc.tile_pool(name=f"dp{_rep}", bufs=2) as dpool, \
                     tc.tile_pool(name=f"dq{_rep}", bufs=1) as dqool, \
                     tc.tile_pool(name=f"scp{_rep}", bufs=2,
                                  space="PSUM") as scpsum:
                    if "decode" not in ablate:
                        u_sb = dqool.tile([P, NE16], f32, tag="ut")
                        v_sb = dqool.tile([P, NE16], f32, tag="vt")
                        uvv = uvtab[:].rearrange("(c q) s j -> q s c j", q=2)
                        for g in range(NGRP):
                            nc.sync.dma_start(
                                out=u_sb[16 * g:16 * (g + 1), :]
                                .rearrange("s (c j) -> s c j", j=NJ),
                                in_=uvv[0])
                            nc.sync.dma_start(
                                out=v_sb[16 * g:16 * (g + 1), :]
                                .rearrange("s (c j) -> s c j", j=NJ),
                                in_=uvv[1])
                        for ki, kind in enumerate(("pos", "neg")):
                            uit, umk = didx_t[(kind, "u")]
                            vit, vmk = didx_t[(kind, "v")]
                            Gu = dpool.tile([P, NDEC, 1], f32, tag="gu")
                            nc.gpsimd.ap_gather(
                                out_ap=Gu[:], in_ap=u_sb[:].unsqueeze(2),
                                idxs_ap=uit[:], channels=P,
                                num_elems=NE16, d=1, num_idxs=NDEC)
                            Gv = dpool.tile([P, NDEC, 1], f32, tag="gv")
                            nc.gpsimd.ap_gather(
                                out_ap=Gv[:], in_ap=v_sb[:].unsqueeze(2),
                                idxs_ap=vit[:], channels=P,
                                num_elems=NE16, d=1, num_idxs=NDEC)
                            t1 = dpool.tile([P, NDEC], bf16, tag="t1d")
                            nc.vector.tensor_tensor(
                                out=t1[:], in0=Gu[:, :, 0], in1=umk[:],
                                op=Mult)
                            t2 = dpool.tile([P, NDEC], bf16, tag="t2d")
                            nc.vector.tensor_tensor(
                                out=t2[:], in0=Gv[:, :, 0], in1=vmk[:],
                                op=Mult)
                            Gs = dpool.tile([P, NDEC], bf16, tag="gs")
                            nc.vector.tensor_tensor(
                                out=Gs[:], in0=t1[:], in1=t2[:], op=Add)
                            for j0 in range(0, NDEC, 512):
                                n = min(512, NDEC - j0)
                                scp = scpsum.tile([NGRP, 512], f32,
                                                  space="PSUM", tag="scp")
                                nc.tensor.matmul(out=scp[:, :n],
                                                 lhsT=gselb[:],
                                                 rhs=Gs[:, j0:j0 + n],
                                                 start=True, stop=True)
                                sc1 = dpool.tile([NGRP, 512], f32,
                                                 tag="sc1")
                                nc.vector.tensor_scalar(
                                    out=sc1[:, :n], in0=scp[:, :n],
                                    scalar1=Cb[:, 0:1], scalar2=None,
                                    op0=Add)
                                nc.sync.dma_start(
                                    out=scored[ki * NGRP:(ki + 1) * NGRP,
                                               j0:j0 + n],
                                    in_=sc1[:, :n])

    nc.compile()
    return nc


def assemble(meta, score_arrs):
    book = meta["book"]
    out = {}
    for ki, (kind, total) in enumerate((("pos", meta["npos"]),
                                        ("neg", meta["nneg"]))):
        sc = np.empty(total, np.float32)
        for c in range(NCORE):
            arr = score_arrs[c].reshape(2 * NGRP, NDEC)
            bk = book[(kind, c)]
            valid = bk >= 0
            sc[bk[valid]] = arr[ki * NGRP:(ki + 1) * NGRP][valid]
        out[kind] = sc
    return out["pos"], out["neg"]


_CACHE = {}


def kernel(**inputs):
    meta, in_maps = prepare(inputs)
    key = (meta["S"], meta["Wdeg"], tuple(meta["K"].ravel()))
    if key not in _CACHE:
        _CACHE[key] = build(meta)
    nc = _CACHE[key]

    from concourse.bass_utils import run_bass_kernel_spmd
    res = run_bass_kernel_spmd(nc, in_maps, core_ids=list(range(NCORE)))
    return assemble(meta, [np.asarray(r["scores"]).reshape(-1)
                           for r in res.results])

